# revision 17
# baseline (speedup 1.0000x reference)
"""Trainium2 Bass kernel for CategoryConstrainedGRUMN (2-layer GRU with memory-tape
attention + category conditioning + vocab projection).

Strategy (pure SPMD across 8 NeuronCores, no collectives):
  core i handles batch b=i%4 and vocab half hv=i//4.
  - on-device embedding gather (indirect DMA) + transposed precomputes
  - 256 fully-unrolled recurrent steps, state in column-major [128, chunks] layout
  - weights-stationary matmuls (bf16) produce column outputs directly
  - softmax kept unnormalized: exp -> sum via matmul -> reciprocal -> per-partition
    broadcast via rank-1 matmul -> folded into the agg evacuation (tensor_scalar)
  - sigmoid via tanh (keeps ScalarE on one LUT table set) + affine
  - output phase: ctx/logits as big batched matmuls over the collected tape
Cores sharing a batch duplicate the recurrence (idle otherwise); each core emits
logits[b, :, hv*16000:(hv+1)*16000]. Host reassembles (logits, h0, h1).
"""

import numpy as np
import ml_dtypes

import concourse.bass as bass
import concourse.bacc as bacc
import concourse.mybir as mybir
import concourse.tile as tile
from concourse.bass_utils import run_bass_kernel_spmd

BF16 = ml_dtypes.bfloat16
bf = mybir.dt.bfloat16
f32 = mybir.dt.float32
i32 = mybir.dt.int32

B, T, E, H, A, V, C = 4, 256, 256, 256, 256, 32000, 100
P = 128
VH = V // 2          # vocab half per core
NB = VH // 500       # 32 output column blocks of width 500
AF = mybir.ActivationFunctionType
ALU = mybir.AluOpType

IN_SPECS = [
    # gather indices + tables
    ("ids_we", [P, 2], i32), ("ids_ce", [P, 2], i32),
    ("emb_bf", [V, E], bf), ("cemb_bf", [C, E], bf),
    # stationary weights [128, Kc, M] (bf16)
    ("Wih0s", [P, 4, 768], bf), ("Wx0s", [P, 4, 256], bf),
    ("Wh0s", [P, 2, 256], bf), ("Wth0s", [P, 2, 256], bf), ("Whh0s", [P, 2, 768], bf),
    ("Wih1s", [P, 2, 768], bf), ("Wx1s", [P, 2, 256], bf),
    ("Wh1s", [P, 2, 256], bf), ("Wth1s", [P, 2, 256], bf), ("Whh1s", [P, 2, 768], bf),
    ("Wctxs", [P, 4, 500], bf),
    ("v0c", [P, 2], bf), ("v1c", [P, 2], bf),
    # psum-init bias matrices (rows = output chunks) bf16
    ("bhn0m", [2, P], bf), ("bru1m", [4, P], bf), ("bnh1m", [4, P], bf), ("bq1m", [2, P], bf),
    # f32 bias columns
    ("bfold0c", [P, 6], f32), ("c0c", [P, 2], f32), ("bctxc", [P, 4], f32),
    # constants
    ("I4", [4, 4], bf), ("ones_r_bf", [1, P], bf), ("ones_r_f32", [1, P], f32),
    ("ones_c_bf", [P, 1], bf), ("ident", [P, P], bf),
    # output projection
    ("WoutS", [512, VH], bf), ("boutR", [1, VH], bf),
]


def build_nc(t_steps=T):
    nc = bacc.Bacc("TRN2", target_bir_lowering=False, debug=False)

    dins = {n: nc.dram_tensor(n, shp, dt, kind="ExternalInput") for n, shp, dt in IN_SPECS}
    d_logits = nc.dram_tensor("logits", [T, VH], f32, kind="ExternalOutput")
    d_houts = nc.dram_tensor("houts", [2, T], f32, kind="ExternalOutput")

    with tile.TileContext(nc) as tc:
        with (
            tc.tile_pool(name="res", bufs=1) as res,        # persistent SBUF
            tc.tile_pool(name="wk", bufs=2) as wk,           # per-step SBUF scratch
            tc.tile_pool(name="ps", bufs=1, space="PSUM") as ps,
            tc.tile_pool(name="ps2", bufs=2, space="PSUM") as ps2,
        ):
            # ---------- load resident tensors ----------
            sb = {}
            for n, shp, dt in IN_SPECS:
                if n in ("emb_bf", "cemb_bf", "WoutS"):
                    continue
                sb[n] = res.tile(shp, dt, tag=n, name=n)
                nc.sync.dma_start(sb[n][:], dins[n][:])
            wsrc = dins["WoutS"].ap().rearrange("(c p) n -> p c n", p=P)

            # ---------- persistent state ----------
            TH_T = [res.tile([P, 2, T], f32, tag=f"th_t{l}", name=f"th_t{l}") for l in range(2)]
            TAPE = [res.tile([P, 2, T], bf, tag=f"tape{l}", name=f"tape{l}") for l in range(2)]
            GI0_T = res.tile([P, 6, T], f32, tag="gi0t")
            XQ0_T = res.tile([P, 2, T], f32, tag="xq0t")
            CE_T = res.tile([P, 2, T], bf, tag="cet")
            X0_T = res.tile([P, 4, T], bf, tag="x0t")
            NH1_T = res.tile([P, 2, T], bf, tag="nh1t")
            CTX_T = res.tile([P, 4, T], bf, tag="ctxt")
            h_init = res.tile([P, 2], bf, tag="h_init")

            for l in range(2):
                nc.vector.memset(TAPE[l][:], 0.0)
            nc.vector.memset(CTX_T[:], 0.0)
            nc.vector.memset(h_init[:], 0.0)

            # ---------- embedding gather + transposes ----------
            rows = {}
            for nm, tbl, idx in (("we", "emb_bf", "ids_we"), ("ce", "cemb_bf", "ids_ce")):
                r = res.tile([P, 2, E], bf, tag=f"{nm}_rows", name=f"{nm}_rows")
                rows[nm] = r
                for c in range(2):
                    nc.gpsimd.indirect_dma_start(
                        out=r[:, c, :], out_offset=None,
                        in_=dins[tbl][:],
                        in_offset=bass.IndirectOffsetOnAxis(ap=sb[idx][:, c : c + 1], axis=0),
                    )
            # transpose ce rows -> col-major CE_T [E(part), chunk, t]
            for tcb in range(2):
                for ec in range(2):
                    pt = ps2.tile([P, P], bf, tag="big")
                    nc.tensor.transpose(pt[:], rows["ce"][:, tcb, ec * P : (ec + 1) * P], sb["ident"][:])
                    nc.vector.tensor_copy(CE_T[:, ec, tcb * P : (tcb + 1) * P], pt[:])
            ce_T = CE_T
            # X0_T = [relu(we_T + ce_T); relu(2*ce_T)]  (we_T blocks stay in psum)
            for tcb in range(2):
                for ec in range(2):
                    pt = ps2.tile([P, P], bf, tag="big")
                    nc.tensor.transpose(pt[:], rows["we"][:, tcb, ec * P : (ec + 1) * P], sb["ident"][:])
                    s = wk.tile([P, P], f32, tag="x0tmp")
                    nc.vector.tensor_add(s[:], pt[:], ce_T[:, ec, tcb * P : (tcb + 1) * P])
                    nc.scalar.activation(X0_T[:, ec, tcb * P : (tcb + 1) * P], s[:], AF.Relu)
            for ec in range(2):
                nc.scalar.activation(X0_T[:, 2 + ec, :], ce_T[:, ec, :], AF.Relu, scale=2.0)

            # ---------- GI0_T / XQ0_T precompute ----------
            for m in range(6):
                pg = ps2.tile([P, T], f32, tag="big")
                for kc in range(4):
                    nc.tensor.matmul(pg[:], sb["Wih0s"][:, kc, m * P : (m + 1) * P], X0_T[:, kc, :],
                                     start=(kc == 0), stop=(kc == 3))
                if m < 4:
                    nc.vector.tensor_scalar(GI0_T[:, m, :], pg[:], sb["bfold0c"][:, m : m + 1], None, ALU.add)
                else:  # n-gate input pre-doubled for the fused gate math
                    nc.vector.tensor_scalar(GI0_T[:, m, :], pg[:], sb["bfold0c"][:, m : m + 1], 2.0, ALU.add, ALU.mult)
            for m in range(2):
                pg = ps2.tile([P, T], f32, tag="big")
                for kc in range(4):
                    nc.tensor.matmul(pg[:], sb["Wx0s"][:, kc, m * P : (m + 1) * P], X0_T[:, kc, :],
                                     start=(kc == 0), stop=(kc == 3))
                nc.vector.tensor_scalar(XQ0_T[:, m, :], pg[:], sb["c0c"][:, m : m + 1], None, ALU.add)

            # ---------- recurrence ----------
            def attention(l, t, q_sb, v_col):
                """returns agg_sb [P,2] bf16 (normalized attention-weighted tape sum)"""
                n_tc = 1 if t <= P else 2
                att = wk.tile([P, 2, T], bf, tag=f"att{l}")
                for a in range(2):
                    nc.scalar.activation(att[:, a, 0:t], TH_T[l][:, a, 0:t], AF.Tanh,
                                         bias=q_sb[:, a : a + 1])
                sA = ps.tile([P, 3], f32, tag="smax")
                sB = ps.tile([P, 3], f32, tag="smax2")
                sc = [sA, sB]
                for tcb in range(n_tc):
                    tl = min(P, t - tcb * P)
                    for a in range(2):
                        nc.tensor.matmul(sc[tcb][0:tl, 0:1],
                                         att[:, a, tcb * P : tcb * P + tl],
                                         v_col[:, a : a + 1],
                                         start=(a == 0), stop=(a == 1))
                expt = wk.tile([P, 2], bf, tag="exp")
                for tcb in range(n_tc):
                    tl = min(P, t - tcb * P)
                    nc.scalar.activation(expt[0:tl, tcb : tcb + 1], sc[tcb][0:tl, 0:1], AF.Exp)
                # S = sum(exp) into sA col 2 (fresh group; exp already consumed sA col 0
                # via the expt true-dependency)
                for tcb in range(n_tc):
                    tl = min(P, t - tcb * P)
                    nc.tensor.matmul(sA[0:1, 2:3], expt[0:tl, tcb : tcb + 1],
                                     sb["ones_c_bf"][0:tl, :], start=(tcb == 0), stop=(tcb == n_tc - 1))
                rec = wk.tile([1, 1], f32, tag="rec")
                nc.vector.reciprocal(rec[:], sA[0:1, 2:3])
                prc = ps.tile([P, 1], f32, tag="rc")
                nc.tensor.matmul(prc[:], sb["ones_r_f32"][:], rec[:], start=True, stop=True)
                rec_col = wk.tile([P, 1], f32, tag="rec_col")
                nc.scalar.copy(rec_col[:], prc[:])
                # agg into sB cols 0:2 (sB's score group is closed; ordered after the
                # chunk-1 exp read via the expt dependency)
                pagg = sB[:, 0:2]
                for tcb in range(n_tc):
                    tl = min(P, t - tcb * P)
                    for m in range(2):
                        nc.tensor.matmul(pagg[:, m : m + 1],
                                         TAPE[l][0:tl, tcb, m * P : (m + 1) * P],
                                         expt[0:tl, tcb : tcb + 1],
                                         start=(tcb == 0 and m == 0),
                                         stop=(tcb == n_tc - 1 and m == 1))
                agg_sb = wk.tile([P, 2], bf, tag="agg_sb")
                nc.vector.tensor_scalar(agg_sb[:], pagg[:], rec_col[:], None, ALU.mult)
                return agg_sb

            def gru_tail(l, t, ru_ps, inew2x_ap, hn_ap, agg_sb):
                """gate math with sigmoid-via-tanh fused:
                r,u = 0.5*(tanh(0.5*ru_in)+1); n = tanh(0.5*((tr+1)*hn + 2*inew));
                h' = 0.5*(tu+1)*(agg-n) + n.  inew2x_ap must hold 2*inew."""
                rut = wk.tile([P, 4], f32, tag="rut")
                if l == 0:
                    nc.vector.tensor_add(rut[:], inew2x_ap[1], ru_ps)  # GI0_ru + gh_ru
                    nc.scalar.activation(rut[:], rut[:], AF.Tanh, scale=0.5)
                else:
                    nc.scalar.activation(rut[:], ru_ps, AF.Tanh, scale=0.5)
                rn = wk.tile([P, 2], f32, tag="rn")
                nc.vector.scalar_tensor_tensor(rn[:], rut[:, 0:2], 1.0, hn_ap, ALU.add, ALU.mult)
                nin = wk.tile([P, 2], f32, tag="nin")
                nc.vector.tensor_add(nin[:], rn[:], inew2x_ap[0])
                nt = wk.tile([P, 2], f32, tag="nt")
                nc.scalar.activation(nt[:], nin[:], AF.Tanh, scale=0.5)
                d = wk.tile([P, 2], f32, tag="d")
                nc.vector.tensor_sub(d[:], agg_sb[:], nt[:])
                ud = wk.tile([P, 2], f32, tag="ud")
                nc.vector.scalar_tensor_tensor(ud[:], rut[:, 2:4], 1.0, d[:], ALU.add, ALU.mult)
                hn_new = wk.tile([P, 2], bf, tag=f"h{l}")
                nc.vector.scalar_tensor_tensor(hn_new[:], ud[:], 0.5, nt[:], ALU.mult, ALU.add)
                return hn_new

            def store_h(l, t, h_new):
                for c in range(2):
                    nc.sync.dma_start(
                        TAPE[l][t % P : t % P + 1, t // P, c * P : (c + 1) * P],
                        h_new[:, c : c + 1])

            def layer0(t, h_prev):
                # th0 | hq0
                pth = ps.tile([P, 4], f32, tag="th")
                for kc in range(2):
                    for m in range(2):
                        nc.tensor.matmul(pth[:, m : m + 1], sb["Wh0s"][:, kc, m * P : (m + 1) * P],
                                         h_prev[:, kc : kc + 1],
                                         start=(kc == 0 and m == 0), stop=False)
                        nc.tensor.matmul(pth[:, 2 + m : 3 + m], sb["Wth0s"][:, kc, m * P : (m + 1) * P],
                                         h_prev[:, kc : kc + 1],
                                         start=False, stop=(kc == 1 and m == 1))
                nc.vector.tensor_copy(TH_T[0][:, :, t], pth[:, 0:2])
                q0 = wk.tile([P, 2], f32, tag="q0")
                nc.vector.tensor_add(q0[:], XQ0_T[:, :, t], pth[:, 2:4])
                if t > 0:
                    agg_sb = attention(0, t, q0, sb["v0c"])
                else:
                    agg_sb = h_init
                pru = ps.tile([P, 4], f32, tag="ru")
                phn = ps.tile([P, 4], f32, tag="hn")
                nc.tensor.matmul(phn[:, 0:2], sb["bhn0m"][:], sb["I4"][0:2, 0:2], start=True, stop=False)
                for kc in range(2):
                    for m in range(6):
                        dst = pru[:, m : m + 1] if m < 4 else phn[:, m - 4 : m - 3]
                        nc.tensor.matmul(dst, sb["Whh0s"][:, kc, m * P : (m + 1) * P],
                                         agg_sb[:, kc : kc + 1],
                                         start=(kc == 0 and m == 0),
                                         stop=(kc == 1 and (m == 3 or m == 5)))
                return gru_tail(0, t, pru[:, 0:4],
                                (GI0_T[:, 4:6, t], GI0_T[:, 0:4, t]), phn[:, 0:2], agg_sb)

            def layer1(t, h_prev, nh0):
                li1s = wk.tile([P, 2], f32, tag="li1s")
                nc.vector.tensor_add(li1s[:], nh0[:], CE_T[:, :, t])
                li1 = wk.tile([P, 2], bf, tag="li1")
                nc.scalar.activation(li1[:], li1s[:], AF.Relu)
                # th1 / q1(=C1+xq1+hq1)
                pth = ps.tile([P, 4], f32, tag="th")
                pq1 = ps.tile([P, 3], f32, tag="smax2")
                nc.tensor.matmul(pq1[:, 0:2], sb["bq1m"][:], sb["I4"][0:2, 0:2], start=True, stop=False)
                for kc in range(2):
                    for m in range(2):
                        nc.tensor.matmul(pth[:, m : m + 1], sb["Wh1s"][:, kc, m * P : (m + 1) * P],
                                         h_prev[:, kc : kc + 1],
                                         start=(kc == 0 and m == 0), stop=(kc == 1 and m == 1))
                        nc.tensor.matmul(pq1[:, m : m + 1], sb["Wx1s"][:, kc, m * P : (m + 1) * P],
                                         li1[:, kc : kc + 1], start=False, stop=False)
                        nc.tensor.matmul(pq1[:, m : m + 1], sb["Wth1s"][:, kc, m * P : (m + 1) * P],
                                         h_prev[:, kc : kc + 1],
                                         start=False, stop=(kc == 1 and m == 1))
                nc.vector.tensor_copy(TH_T[1][:, :, t], pth[:, 0:2])
                q1 = wk.tile([P, 2], f32, tag="q0")
                nc.scalar.copy(q1[:], pq1[:, 0:2])
                if t > 0:
                    agg_sb = attention(1, t, q1, sb["v1c"])
                else:
                    agg_sb = h_init
                pru = ps.tile([P, 4], f32, tag="ru")
                pnh = ps.tile([P, 4], f32, tag="hn")
                nc.tensor.matmul(pru[:], sb["bru1m"][:], sb["I4"][:], start=True, stop=False)
                nc.tensor.matmul(pnh[:], sb["bnh1m"][:], sb["I4"][:], start=True, stop=False)
                for kc in range(2):
                    for m in range(6):
                        dst = pru[:, m : m + 1] if m < 4 else pnh[:, m - 4 : m - 3]
                        nc.tensor.matmul(dst, sb["Wih1s"][:, kc, m * P : (m + 1) * P],
                                         li1[:, kc : kc + 1], start=False, stop=False)
                for kc in range(2):
                    for m in range(6):
                        dst = pru[:, m : m + 1] if m < 4 else pnh[:, 2 + m - 4 : 3 + m - 4]
                        nc.tensor.matmul(dst, sb["Whh1s"][:, kc, m * P : (m + 1) * P],
                                         agg_sb[:, kc : kc + 1],
                                         start=False, stop=(kc == 1 and (m == 3 or m == 5)))
                return gru_tail(1, t, pru[:], (pnh[:, 0:2], None), pnh[:, 2:4], agg_sb)

            h0c, h1c = h_init, h_init
            for t in range(t_steps):
                nh0 = layer0(t, h0c)
                store_h(0, t, nh0)
                nh1 = layer1(t, h1c, nh0)
                store_h(1, t, nh1)
                h0c, h1c = nh0, nh1

            # ---------- final hidden states ----------
            last_p, last_c = (t_steps - 1) % P, (t_steps - 1) // P
            for l in range(2):
                hbf = wk.tile([1, T], bf, tag="hbf", name=f"hbf{l}")
                nc.sync.dma_start(hbf[:], TAPE[l][last_p : last_p + 1, last_c, :])
                hst = wk.tile([P, 500], f32, tag="lg", name=f"hst{l}")
                nc.vector.tensor_copy(hst[0:1, 0:T], hbf[:])
                nc.sync.dma_start(d_houts[l : l + 1, :], hst[0:1, 0:T])

            # ---------- output phase ----------
            for tcb in range(2):
                for hc in range(2):
                    pt = ps2.tile([P, P], bf, tag="big")
                    nc.tensor.transpose(pt[:], TAPE[1][:, tcb, hc * P : (hc + 1) * P], sb["ident"][:])
                    nc.vector.tensor_copy(NH1_T[:, hc, tcb * P : (tcb + 1) * P], pt[:])
            for m in range(4):
                ml = min(P, 500 - m * P)
                pc = ps2.tile([P, T], f32, tag="big")
                for kc in range(4):
                    rhs = NH1_T[:, kc, :] if kc < 2 else CE_T[:, kc - 2, :]
                    nc.tensor.matmul(pc[0:ml, :], sb["Wctxs"][:, kc, m * P : m * P + ml], rhs,
                                     start=(kc == 0), stop=(kc == 3))
                nc.scalar.activation(CTX_T[0:ml, m, :], pc[0:ml, :], AF.Tanh,
                                     bias=sb["bctxc"][0:ml, m : m + 1])
            for nb in range(NB):
                wo = res.tile([P, 4, 500], bf, tag="wout", bufs=16, name=f"wo{nb}")
                nc.sync.dma_start(wo[:], wsrc[:, :, nb * 500 : (nb + 1) * 500])
                for tcb in range(2):
                    pl = ps2.tile([P, 500], f32, tag="big")
                    nc.tensor.matmul(pl[:], sb["ones_r_bf"][:], sb["boutR"][:, nb * 500 : (nb + 1) * 500],
                                     start=True, stop=False)
                    for kc in range(4):
                        nc.tensor.matmul(pl[:], CTX_T[:, kc, tcb * P : (tcb + 1) * P],
                                         wo[:, kc, :],
                                         start=False, stop=(kc == 3))
                    lg = wk.tile([P, 500], f32, tag="lg")
                    if tcb == 0:
                        nc.scalar.copy(lg[:], pl[:])
                    else:
                        nc.vector.tensor_copy(lg[:], pl[:])
                    nc.sync.dma_start(d_logits[tcb * P : (tcb + 1) * P, nb * 500 : (nb + 1) * 500], lg[:])

    nc.compile()
    return nc


def make_core_inputs(ins, b, hv):
    def st(w, kc):  # stationary rearrange [K,M] -> [128, Kc, M] bf16
        k, m = w.shape
        return np.ascontiguousarray(w.reshape(kc, P, m).transpose(1, 0, 2)).astype(BF16)

    g = lambda n: np.asarray(ins[n], np.float32)
    d = {}
    d["ids_we"] = np.ascontiguousarray(np.asarray(ins["input_ids"][b]).astype(np.int32).reshape(2, P).T)
    d["ids_ce"] = np.ascontiguousarray(np.asarray(ins["category_ids"][b]).astype(np.int32).reshape(2, P).T)
    d["emb_bf"] = g("emb").astype(BF16)
    d["cemb_bf"] = g("cat_emb").astype(BF16)
    for l, kc in ((0, 4), (1, 2)):
        wih = g(f"Wih{l}")
        if l == 1:  # n-gate input pre-doubled for the fused gate math
            wih = wih.copy()
            wih[:, 2 * H :] *= 2.0
        d[f"Wih{l}s"] = st(wih, kc)
        d[f"Wx{l}s"] = st(g(f"Wx{l}"), kc)
        d[f"Wh{l}s"] = st(g(f"Wh{l}"), 2)
        d[f"Wth{l}s"] = st(g(f"Wth{l}"), 2)
        d[f"Whh{l}s"] = st(g(f"Whh{l}"), 2)
        d[f"v{l}c"] = np.ascontiguousarray(g(f"v{l}").reshape(2, P).T).astype(BF16)
    d["Wctxs"] = st(g("Wctx"), 4)
    d["bhn0m"] = g("bhh0")[2 * H :].reshape(2, P).astype(BF16)
    d["bru1m"] = (g("bih1") + g("bhh1"))[: 2 * H].reshape(4, P).astype(BF16)
    d["bnh1m"] = np.concatenate([2.0 * g("bih1")[2 * H :], g("bhh1")[2 * H :]]).reshape(4, P).astype(BF16)
    d["bq1m"] = (g("bx1") + g("bh1") + g("bth1")).reshape(2, P).astype(BF16)
    bf0 = g("bih0") + np.concatenate([g("bhh0")[: 2 * H], np.zeros(H, np.float32)])
    d["bfold0c"] = np.ascontiguousarray(bf0.reshape(6, P).T)
    d["c0c"] = np.ascontiguousarray((g("bx0") + g("bh0") + g("bth0")).reshape(2, P).T)
    d["bctxc"] = np.ascontiguousarray(np.pad(g("bctx"), (0, 12)).reshape(4, P).T)
    d["I4"] = np.eye(4, dtype=BF16)
    d["ones_r_bf"] = np.ones((1, P), BF16)
    d["ones_r_f32"] = np.ones((1, P), np.float32)
    d["ones_c_bf"] = np.ones((P, 1), BF16)
    d["ident"] = np.eye(P, dtype=BF16)
    d["WoutS"] = np.pad(g("Wout")[:, hv * VH : (hv + 1) * VH], ((0, 12), (0, 0))).astype(BF16)
    d["boutR"] = np.ascontiguousarray(g("bout")[hv * VH : (hv + 1) * VH].reshape(1, VH)).astype(BF16)
    return d


def kernel(**inputs):
    ins = {k: np.asarray(v) for k, v in inputs.items()}
    in_maps = [make_core_inputs(ins, i % 4, i // 4) for i in range(8)]
    nc = build_nc()
    res = run_bass_kernel_spmd(nc, in_maps, core_ids=list(range(8)))
    logits = np.zeros((B, T, V), np.float32)
    h0 = np.zeros((B, H), np.float32)
    h1 = np.zeros((B, H), np.float32)
    for i in range(8):
        b, hv = i % 4, i // 4
        r = res.results[i]
        logits[b, :, hv * VH : (hv + 1) * VH] = r["logits"]
        if hv == 0:
            h0[b] = r["houts"][0]
            h1[b] = r["houts"][1]
    return logits, h0, h1
